# revision 27
# baseline (speedup 1.0000x reference)
"""HarmonicCausalSelfAttention on 8 TRN2 NeuronCores.

Sharding: core c -> (batch b = c//2, head-group g = c%2); each core computes
attention for 8 heads of one batch and a full-width partial of the output
projection; the host sums the two partials per batch (the rank-128 c-proj
intermediate is linear, so out = (r_g0 + r_g1) @ cB^T = part_g0 + part_g1).

v2 layout strategy (everything transposed; no on-device transposes):
  stage1:  t^T[rank, T]   = A @ x^T            (x^T chunk-DMA'd, bf16)
  stage2:  Q^T/K^T stored as head-PAIR tiles [128, T] (head 2hp rows 0:64,
           head 2hp+1 rows 64:128) -> M=128 matmuls; V[keys, 64+ones]
  attn:    S^T[keys, q] = K_kb @ Q^T  (causal-trimmed strips, 2-bank tiles)
           P^T = exp(S^T * 1/8)  on ScalarE (PSUM -> SBUF bf16)
           PV:  psum[65, 512q] += [V_kb | 1]^T @ P^T_kb  (row 64 = denom)
  norm:    reciprocal_approx_fast(denom row) -> rec[97-row tile]
           ones-matmul broadcast (K=1) -> bf16 cast -> in-place DVE multiply
           (deferred off the exp critical path; no slow RECIPROCAL, no
           per-strip broadcast chains)
  c_proj:  r^T[rank, T] = sum_hp cAT2_hp @ Y2^T_hp (K=128 pairs); out chunks
           cast bf16 and DMA'd (host sums partials in f32)
"""

import numpy as np
import ml_dtypes

import concourse.bass as bass
from concourse import bacc
import concourse.mybir as mybir
from concourse.tile import TileContext
from concourse.bass_utils import run_bass_kernel_spmd

B, T, C = 4, 2048, 1024
NH, HD = 16, 64
RANK = 128
NCORES = 8
HPC = 8          # heads per core
G = 512          # C columns per head group
P = 128
F32 = mybir.dt.float32
BF16 = mybir.dt.bfloat16
BF = ml_dtypes.bfloat16

_NC_CACHE = None


def _chunks(total, step):
    res = []
    o = 0
    while o < total:
        res.append((o, min(step, total - o)))
        o += min(step, total - o)
    return res


def _scalar_recip(nc, out_ap, in_ap):
    sc = nc.scalar
    return sc.add_instruction(mybir.InstActivation(
        name=sc.bass.get_next_instruction_name(),
        func=mybir.ActivationFunctionType.Reciprocal,
        ins=[
            sc.lower_ap(in_ap),
            mybir.ImmediateValue(dtype=mybir.dt.float32, value=0.0),
            mybir.ImmediateValue(dtype=mybir.dt.float32, value=1.0),
            mybir.ImmediateValue(dtype=mybir.dt.float32, value=0.0),
        ],
        outs=[sc.lower_ap(out_ap)],
    ))


def build():
    nc = bacc.Bacc()
    dp = nc.declare_dram_parameter
    xT = dp("xT", [C, T], BF16, isOutput=False)
    qAT = dp("qAT", [C, RANK], BF16, isOutput=False)
    kAT = dp("kAT", [C, RANK], BF16, isOutput=False)
    vAT = dp("vAT", [C, RANK], BF16, isOutput=False)
    qBT = dp("qBT", [RANK, G], BF16, isOutput=False)
    kBT = dp("kBT", [RANK, G], BF16, isOutput=False)
    vBT = dp("vBT", [RANK, G], BF16, isOutput=False)
    cAT = dp("cAT", [G, RANK], BF16, isOutput=False)
    cBT = dp("cBT", [RANK, C], BF16, isOutput=False)
    maskp = dp("mask", [P, P], BF16, isOutput=False)
    out = dp("out", [T, C], BF16, isOutput=True)

    Exp = mybir.ActivationFunctionType.Exp
    ADD = mybir.AluOpType.add

    with TileContext(nc) as tc:
        with tc.tile_pool(name="sb", bufs=1) as sb:
            warm_sb = sb.tile([64, 512], BF16, tag="warm")
            nc.gpsimd.memset(warm_sb, 1.0)
            mask_sb = sb.tile([P, P], BF16, tag="mask")
            nc.gpsimd.dma_start(out=mask_sb, in_=maskp[:, :])
            qAT_sb = sb.tile([P, 8, RANK], BF16, tag="qAT")
            nc.gpsimd.dma_start(out=qAT_sb, in_=qAT.rearrange("(co ci) r -> ci co r", ci=P))
            kAT_sb = sb.tile([P, 8, RANK], BF16, tag="kAT")
            nc.gpsimd.dma_start(out=kAT_sb, in_=kAT.rearrange("(co ci) r -> ci co r", ci=P))
            vAT_sb = sb.tile([P, 8, RANK], BF16, tag="vAT")
            nc.sync.dma_start(out=vAT_sb, in_=vAT.rearrange("(co ci) r -> ci co r", ci=P))
            qBT_sb = sb.tile([RANK, G], BF16, tag="qBT")
            nc.sync.dma_start(out=qBT_sb, in_=qBT[:, :])
            kBT_sb = sb.tile([RANK, G], BF16, tag="kBT")
            nc.sync.dma_start(out=kBT_sb, in_=kBT[:, :])
            vBT_sb = sb.tile([RANK, G], BF16, tag="vBT")
            nc.sync.dma_start(out=vBT_sb, in_=vBT[:, :])
            cAT2_sb = sb.tile([P, 4, RANK], BF16, tag="cAT")
            nc.sync.dma_start(out=cAT2_sb, in_=cAT.rearrange("(hp d) r -> d hp r", d=P))
            cBT_sb = sb.tile([RANK, C], BF16, tag="cBT")
            nc.sync.dma_start(out=cBT_sb, in_=cBT[:, :])
            xT_sb = sb.tile([P, 8, T], BF16, tag="xT")
            xT_r = xT.rearrange("(co ci) t -> ci co t", ci=P)
            xqs = [nc.gpsimd, nc.sync, nc.scalar]
            for cc in range(8):
                xqs[cc % 3].dma_start(out=xT_sb[:, cc, :], in_=xT_r[:, cc, :])

            # exp act-table preload: tiny dummy activation right after mask DMA
            scr = sb.tile([1, 8], F32, tag="scr")
            nc.scalar.activation(scr, mask_sb[0:1, 0:8], Exp)

            QT2 = [sb.tile([P, T], BF16, tag=f"QT{p}", name=f"QT{p}") for p in range(4)]
            KT2 = [sb.tile([P, T], BF16, tag=f"KT{p}", name=f"KT{p}") for p in range(4)]
            Y2 = [sb.tile([P, T], BF16, tag=f"Y{p}", name=f"Y{p}") for p in range(4)]
            V_sb = sb.tile([P, 16, HPC, 65], BF16, tag="Vsb")
            tTq = sb.tile([P, T], BF16, tag="tTq")
            tTk = sb.tile([P, T], BF16, tag="tTk")
            tTv = sb.tile([P, T], BF16, tag="tTv")
            rT_sb = sb.tile([P, T], BF16, tag="rT")
            onesP = sb.tile([65, 64], BF16, tag="onesP")
            den_t = [sb.tile([65, 4096], BF16, tag=f"den{p}", name=f"den{p}")
                     for p in range(4)]

            nc.gpsimd.memset(V_sb[:, :, :, 64:65], 1.0)
            nc.gpsimd.memset(onesP, 1.0)
            # spin the PE during the input-DMA window so HAM un-throttles the
            # clock (1.2 -> 2.4 GHz) before stage 1 begins
            with tc.tile_pool(name="psW", bufs=1, space="PSUM") as psW:
                pw = psW.tile([64, 512], F32, tag="pw")
                for _ in range(12):
                    nc.tensor.matmul(
                        pw, warm_sb[:, 0:64], warm_sb, start=True, stop=True)

            # ---- phase A: t^T = A @ x^T for q,k (interleaved), then v ----
            with tc.tile_pool(name="psA", bufs=2, space="PSUM") as psA:
                ptq = psA.tile([P, T], F32, tag="psA", name="ptq")
                ptk = psA.tile([P, T], F32, tag="psA", name="ptk")
                for cc in range(8):
                    for AT_sb, pt in ((qAT_sb, ptq), (kAT_sb, ptk)):
                        for t0, tw in _chunks(T, 512):
                            nc.tensor.matmul(
                                pt[:, t0:t0 + tw],
                                AT_sb[:, cc, :],
                                xT_sb[:, cc, t0:t0 + tw],
                                start=(cc == 0), stop=(cc == 7),
                            )
                nc.vector.tensor_copy(out=tTq, in_=ptq)
                nc.vector.tensor_copy(out=tTk, in_=ptk)
                ptv = psA.tile([P, T], F32, tag="psA", name="ptv")
                for cc in range(8):
                    for t0, tw in _chunks(1024, 512):
                        nc.tensor.matmul(
                            ptv[:, t0:t0 + tw],
                            vAT_sb[:, cc, :],
                            xT_sb[:, cc, t0:t0 + tw],
                            start=(cc == 0), stop=(cc == 7),
                        )
                nc.vector.tensor_copy(out=tTv[:, 0:1024], in_=ptv[:, 0:1024])

            # ---- phase B: paired Q^T/K^T (M=128) and keys-major V ----
            with (
                tc.tile_pool(name="psB", bufs=2, space="PSUM") as psB,
                tc.tile_pool(name="psV", bufs=2, space="PSUM") as psV,
            ):
                def stage2_pair(hp):
                    for BT_sb, dest, tT in ((qBT_sb, QT2, tTq), (kBT_sb, KT2, tTk)):
                        for t0, tw in _chunks(T, 512):
                            p2 = psB.tile([P, 512], F32, tag="psB")
                            nc.tensor.matmul(
                                p2[:, :tw],
                                BT_sb[:, hp * P:(hp + 1) * P],
                                tT[:, t0:t0 + tw],
                                start=True, stop=True,
                            )
                            nc.vector.tensor_copy(
                                out=dest[hp][:, t0:t0 + tw], in_=p2[:, :tw])

                stage2_pair(0)
                for ti in range(8):
                    pv = psV.tile([P, G], F32, tag="psV")
                    nc.tensor.matmul(
                        pv, tTv[:, ti * 128:(ti + 1) * 128], vBT_sb,
                        start=True, stop=True,
                    )
                    nc.vector.tensor_copy(
                        out=V_sb[:, ti, :, 0:64],
                        in_=pv.rearrange("p (h d) -> p h d", d=64),
                    )

            # ---- attention + deferred normalize ----
            with (
                tc.tile_pool(name="psS", bufs=2, space="PSUM") as psS,
                tc.tile_pool(name="psPV", bufs=3, space="PSUM") as psPV,
                tc.tile_pool(name="ptp", bufs=4) as ptp,
            ):
                strips = [(hp, j, kb)
                          for hp in range(4)
                          for j in range(4)
                          for kb in range(4 * j + 4)]
                pvt = {}
                sps_l = {}

                def stage2_pair_att(hp2):
                    for BT_sb, dest, tT in ((qBT_sb, QT2, tTq), (kBT_sb, KT2, tTk)):
                        for t0, tw in _chunks(T, 512):
                            p2 = psS.tile([P, 512], F32, tag="s2", bufs=1)
                            nc.tensor.matmul(
                                p2[:, :tw],
                                BT_sb[:, hp2 * P:(hp2 + 1) * P],
                                tT[:, t0:t0 + tw],
                                start=True, stop=True,
                            )
                            nc.vector.tensor_copy(
                                out=dest[hp2][:, t0:t0 + tw], in_=p2[:, :tw])

                def v_chunk_att(c):
                    pvc = psS.tile([P, 512], F32, tag="s2", bufs=1, name=f"pvc{c}")
                    for cc in range(8):
                        nc.tensor.matmul(
                            pvc, vAT_sb[:, cc, :],
                            xT_sb[:, cc, 512 * c:512 * (c + 1)],
                            start=(cc == 0), stop=(cc == 7),
                        )
                    nc.vector.tensor_copy(
                        out=tTv[:, 512 * c:512 * (c + 1)], in_=pvc)

                def v_mm_att(ti):
                    pv = psS.tile([P, G], F32, tag="s2", bufs=1, name=f"pvv{ti}")
                    nc.tensor.matmul(
                        pv, tTv[:, ti * 128:(ti + 1) * 128], vBT_sb,
                        start=True, stop=True,
                    )
                    nc.vector.tensor_copy(
                        out=V_sb[:, ti, :, 0:64],
                        in_=pv.rearrange("p (h d) -> p h d", d=64),
                    )

                att_work = {
                    1: lambda: v_chunk_att(2),
                    3: lambda: (v_mm_att(8), v_mm_att(9)),
                    4: lambda: (v_mm_att(10), v_mm_att(11)),
                    6: lambda: v_chunk_att(3),
                    8: lambda: (v_mm_att(12), v_mm_att(13)),
                    9: lambda: (v_mm_att(14), v_mm_att(15)),
                    14: lambda: stage2_pair_att(1),
                    30: lambda: stage2_pair_att(2),
                    46: lambda: stage2_pair_att(3),
                }
                for idx in range(len(strips) + 1):
                    if idx in att_work:
                        att_work[idx]()
                    if idx < len(strips):
                        # emit paired QK for strip idx ahead (row-tiled: both
                        # heads of the pair run concurrently in the PE array)
                        hp, j, kb = strips[idx]
                        c0 = 128 * (kb - 4 * j) if kb >= 4 * j else 0
                        w = 512 - c0
                        qlo = 512 * j + c0
                        sps = psS.tile([P, 2, 512], F32, tag="s")
                        for par in range(2):
                            rws = slice(64 * par, 64 * par + 64)
                            nc.tensor.matmul(
                                sps[:, par, c0:c0 + w],
                                KT2[hp][rws, kb * 128:(kb + 1) * 128],
                                QT2[hp][rws, qlo:qlo + w],
                                start=True, stop=True,
                            )
                        sps_l[idx] = sps
                    if idx == 0:
                        continue
                    hp, j, kb = strips[idx - 1]
                    c0 = 128 * (kb - 4 * j) if kb >= 4 * j else 0
                    w = 512 - c0
                    sps = sps_l.pop(idx - 1)
                    if kb == 0:
                        for par in range(2):
                            pvt[(hp, par, j)] = psPV.tile(
                                [65, 512], F32, tag="pv", name=f"pv{hp}_{par}_{j}")
                    ptile = ptp.tile([P, 2, 512], BF16, tag="pt")
                    nc.scalar.activation(
                        ptile[:, :, c0:], sps[:, :, c0:], Exp, scale=0.125)
                    for par in range(2):
                        h = 2 * hp + par
                        rows = slice(64 * par, 64 * par + 64)
                        if kb >= 4 * j:  # zero masked diag region post-exp
                            # on GpSimd: keeps the mask off the DVE queue so
                            # evac/cast chains can't delay PV -> exp
                            nc.gpsimd.tensor_tensor(
                                out=ptile[:, par, c0:c0 + P],
                                in0=ptile[:, par, c0:c0 + P],
                                in1=mask_sb, op=mybir.AluOpType.mult,
                            )
                        nc.tensor.matmul(
                            pvt[(hp, par, j)][:, c0:],
                            V_sb[:, kb, h, :],
                            ptile[:, par, c0:],
                            start=(kb == 0), stop=(kb == 4 * j + 3),
                        )
                    for par in range(2):
                        if kb == 4 * j + 3:
                            rows = slice(64 * par, 64 * par + 64)
                            r0 = 512 * j
                            pv_t = pvt.pop((hp, par, j))
                            doff = 2048 * par + 512 * j
                            # denominator -> SBUF bf16 (recip deferred to tail)
                            nc.vector.tensor_copy(
                                out=den_t[hp][64:65, doff:doff + 512],
                                in_=pv_t[64:65, :],
                            )
                            # unnormalized Y' -> bf16 SBUF (frees psum)
                            nc.vector.tensor_copy(
                                out=Y2[hp][rows, r0:r0 + 512],
                                in_=pv_t[0:64, :],
                            )

            # ---- normalize tail: ScalarE reciprocal (one table switch),
            # ones-matmul broadcast, DVE multiply ----
            with (
                tc.tile_pool(name="psN", bufs=2, space="PSUM") as psN,
                tc.tile_pool(name="nr2", bufs=3) as nr2,
                tc.tile_pool(name="psD", bufs=1, space="PSUM") as psD,
            ):
                # all 8 reciprocals back-to-back on ScalarE (one table switch)
                rec_l = {}
                for hp in range(4):
                    for par in range(2):
                        doff = 2048 * par
                        rec_t = nr2.tile([65, T], BF16, tag="rect")
                        _scalar_recip(
                            nc, rec_t[64:65, :],
                            den_t[hp][64:65, doff:doff + T],
                        )
                        rec_l[(hp, par)] = rec_t
                pr = psD.tile([P, T], F32, tag="r")
                for hp in range(4):
                    for par in range(2):
                        rows = slice(64 * par, 64 * par + 64)
                        rec_t = rec_l.pop((hp, par))
                        for half in range(2):
                            h0 = 1024 * half
                            bcT = psN.tile([64, 1024], F32, tag="bc", bufs=2)
                            for r0, rw in _chunks(1024, 512):
                                nc.tensor.matmul(
                                    bcT[:, r0:r0 + rw], onesP[64:65, :],
                                    rec_t[64:65, h0 + r0:h0 + r0 + rw],
                                    start=True, stop=True,
                                )
                            nc.vector.tensor_tensor(
                                out=Y2[hp][rows, h0:h0 + 1024],
                                in0=Y2[hp][rows, h0:h0 + 1024],
                                in1=bcT, op=mybir.AluOpType.mult,
                            )
                    # c_proj stage 1 for this pair rides right behind its
                    # normalize (accumulates across pairs in PSUM)
                    for t0, tw in _chunks(T, 512):
                        nc.tensor.matmul(
                            pr[:, t0:t0 + tw], cAT2_sb[:, hp, :], Y2[hp][:, t0:t0 + tw],
                            start=(hp == 0), stop=(hp == 3),
                        )

                nc.vector.tensor_copy(out=rT_sb, in_=pr)

            # ---- phase D2: c_proj output, ti-pair pipelined ----
            out_r = out.rearrange("(tt pp) c -> pp tt c", pp=P)
            with (
                tc.tile_pool(name="psO", bufs=2, space="PSUM") as psO,
                tc.tile_pool(name="ost", bufs=3) as ost,
            ):
                for tp in range(8):
                    po = psO.tile([P, 2, 2, 512], F32, tag="o")
                    for tt in range(2):
                        ti = 2 * tp + tt
                        for nn in range(2):
                            nc.tensor.matmul(
                                po[:, tt, nn, :], rT_sb[:, ti * 128:(ti + 1) * 128],
                                cBT_sb[:, nn * 512:(nn + 1) * 512],
                                start=True, stop=True,
                            )
                    ob = ost.tile([P, 2, C], BF16, tag="ob")
                    if tp % 2 == 0:
                        nc.vector.tensor_copy(out=ob, in_=po)
                    else:
                        nc.scalar.copy(ob, po)
                    q = nc.sync if tp % 2 == 0 else nc.gpsimd
                    q.dma_start(
                        out=out_r[:, 2 * tp:2 * tp + 2, :],
                        in_=ob,
                    )
    nc.finalize()
    return nc


def make_in_maps(x, qA, qB, kA, kB, vA, vB, cA, cB):
    x, qA, qB, kA, kB, vA, vB, cA, cB = [
        np.asarray(a, dtype=np.float32) for a in (x, qA, qB, kA, kB, vA, vB, cA, cB)
    ]
    mask = np.where(
        np.arange(P)[:, None] <= np.arange(P)[None, :], 1.0, 0.0
    ).astype(BF)
    qATn = np.ascontiguousarray(qA.T).astype(BF)
    kATn = np.ascontiguousarray(kA.T).astype(BF)
    vATn = np.ascontiguousarray(vA.T).astype(BF)
    cBTn = np.ascontiguousarray(cB.T).astype(BF)
    in_maps = []
    for c in range(NCORES):
        b, g = divmod(c, 2)
        sl = slice(g * G, (g + 1) * G)
        in_maps.append({
            "xT": np.ascontiguousarray(x[b].T).astype(BF),
            "qAT": qATn, "kAT": kATn, "vAT": vATn,
            "qBT": np.ascontiguousarray(qB[sl, :].T).astype(BF),
            "kBT": np.ascontiguousarray(kB[sl, :].T).astype(BF),
            "vBT": np.ascontiguousarray(vB[sl, :].T).astype(BF),
            "cAT": np.ascontiguousarray(cA[:, sl].T).astype(BF),
            "cBT": cBTn,
            "mask": mask,
        })
    return in_maps


def combine(parts):
    return np.stack(
        [parts[2 * b].astype(np.float32) + parts[2 * b + 1].astype(np.float32)
         for b in range(B)], axis=0)


def kernel(x, qA, qB, kA, kB, vA, vB, cA, cB):
    global _NC_CACHE
    if _NC_CACHE is None:
        _NC_CACHE = build()
    in_maps = make_in_maps(x, qA, qB, kA, kB, vA, vB, cA, cB)
    res = run_bass_kernel_spmd(_NC_CACHE, in_maps, list(range(NCORES))).results
    return combine([res[c]["out"] for c in range(NCORES)])


# revision 28
# speedup vs baseline: 1.0792x; 1.0792x over previous
"""HarmonicCausalSelfAttention on 8 TRN2 NeuronCores.

Sharding: core c -> (batch b = c//2, head-group g = c%2); each core computes
attention for 8 heads of one batch and a full-width partial of the output
projection; the host sums the two partials per batch (the rank-128 c-proj
intermediate is linear, so out = (r_g0 + r_g1) @ cB^T = part_g0 + part_g1).

v2 layout strategy (everything transposed; no on-device transposes):
  stage1:  t^T[rank, T]   = A @ x^T            (x^T chunk-DMA'd, bf16)
  stage2:  Q^T/K^T stored as head-PAIR tiles [128, T] (head 2hp rows 0:64,
           head 2hp+1 rows 64:128) -> M=128 matmuls; V[keys, 64+ones]
  attn:    S^T[keys, q] = K_kb @ Q^T  (causal-trimmed strips, 2-bank tiles)
           P^T = exp(S^T * 1/8)  on ScalarE (PSUM -> SBUF bf16)
           PV:  psum[65, 512q] += [V_kb | 1]^T @ P^T_kb  (row 64 = denom)
  norm:    reciprocal_approx_fast(denom row) -> rec[97-row tile]
           ones-matmul broadcast (K=1) -> bf16 cast -> in-place DVE multiply
           (deferred off the exp critical path; no slow RECIPROCAL, no
           per-strip broadcast chains)
  c_proj:  r^T[rank, T] = sum_hp cAT2_hp @ Y2^T_hp (K=128 pairs); out chunks
           cast bf16 and DMA'd (host sums partials in f32)
"""

import numpy as np
import ml_dtypes

import concourse.bass as bass
from concourse import bacc
import concourse.mybir as mybir
from concourse.tile import TileContext
from concourse.bass_utils import run_bass_kernel_spmd

B, T, C = 4, 2048, 1024
NH, HD = 16, 64
RANK = 128
NCORES = 8
HPC = 8          # heads per core
G = 512          # C columns per head group
P = 128
F32 = mybir.dt.float32
BF16 = mybir.dt.bfloat16
BF = ml_dtypes.bfloat16

_NC_CACHE = None


def _chunks(total, step):
    res = []
    o = 0
    while o < total:
        res.append((o, min(step, total - o)))
        o += min(step, total - o)
    return res


def _scalar_recip(nc, out_ap, in_ap):
    sc = nc.scalar
    return sc.add_instruction(mybir.InstActivation(
        name=sc.bass.get_next_instruction_name(),
        func=mybir.ActivationFunctionType.Reciprocal,
        ins=[
            sc.lower_ap(in_ap),
            mybir.ImmediateValue(dtype=mybir.dt.float32, value=0.0),
            mybir.ImmediateValue(dtype=mybir.dt.float32, value=1.0),
            mybir.ImmediateValue(dtype=mybir.dt.float32, value=0.0),
        ],
        outs=[sc.lower_ap(out_ap)],
    ))


def build():
    nc = bacc.Bacc()
    dp = nc.declare_dram_parameter
    xT = dp("xT", [C, T], BF16, isOutput=False)
    qAT = dp("qAT", [C, RANK], BF16, isOutput=False)
    kAT = dp("kAT", [C, RANK], BF16, isOutput=False)
    vAT = dp("vAT", [C, RANK], BF16, isOutput=False)
    qBT = dp("qBT", [RANK, G], BF16, isOutput=False)
    kBT = dp("kBT", [RANK, G], BF16, isOutput=False)
    vBT = dp("vBT", [RANK, G], BF16, isOutput=False)
    cAT = dp("cAT", [G, RANK], BF16, isOutput=False)
    cBT = dp("cBT", [RANK, C], BF16, isOutput=False)
    maskp = dp("mask", [P, P], BF16, isOutput=False)
    out = dp("out", [T, C], BF16, isOutput=True)

    Exp = mybir.ActivationFunctionType.Exp
    ADD = mybir.AluOpType.add

    with TileContext(nc) as tc:
        with tc.tile_pool(name="sb", bufs=1) as sb:
            warm_sb = sb.tile([64, 512], BF16, tag="warm")
            nc.gpsimd.memset(warm_sb, 1.0)
            mask_sb = sb.tile([P, P], BF16, tag="mask")
            nc.gpsimd.dma_start(out=mask_sb, in_=maskp[:, :])
            qAT_sb = sb.tile([P, 8, RANK], BF16, tag="qAT")
            nc.gpsimd.dma_start(out=qAT_sb, in_=qAT.rearrange("(co ci) r -> ci co r", ci=P))
            kAT_sb = sb.tile([P, 8, RANK], BF16, tag="kAT")
            nc.gpsimd.dma_start(out=kAT_sb, in_=kAT.rearrange("(co ci) r -> ci co r", ci=P))
            vAT_sb = sb.tile([P, 8, RANK], BF16, tag="vAT")
            nc.sync.dma_start(out=vAT_sb, in_=vAT.rearrange("(co ci) r -> ci co r", ci=P))
            qBT_sb = sb.tile([RANK, G], BF16, tag="qBT")
            nc.sync.dma_start(out=qBT_sb, in_=qBT[:, :])
            kBT_sb = sb.tile([RANK, G], BF16, tag="kBT")
            nc.sync.dma_start(out=kBT_sb, in_=kBT[:, :])
            vBT_sb = sb.tile([RANK, G], BF16, tag="vBT")
            nc.sync.dma_start(out=vBT_sb, in_=vBT[:, :])
            cAT2_sb = sb.tile([P, 4, RANK], BF16, tag="cAT")
            nc.sync.dma_start(out=cAT2_sb, in_=cAT.rearrange("(hp d) r -> d hp r", d=P))
            cBT_sb = sb.tile([RANK, C], BF16, tag="cBT")
            nc.sync.dma_start(out=cBT_sb, in_=cBT[:, :])
            xT_sb = sb.tile([P, 8, T], BF16, tag="xT")
            xT_r = xT.rearrange("(co ci) t -> ci co t", ci=P)
            xqs = [nc.gpsimd, nc.sync, nc.scalar]
            for cc in range(8):
                xqs[cc % 3].dma_start(out=xT_sb[:, cc, :], in_=xT_r[:, cc, :])

            # exp act-table preload: tiny dummy activation right after mask DMA
            scr = sb.tile([1, 8], F32, tag="scr")
            nc.scalar.activation(scr, mask_sb[0:1, 0:8], Exp)

            QT2 = [sb.tile([P, T], BF16, tag=f"QT{p}", name=f"QT{p}") for p in range(4)]
            KT2 = [sb.tile([P, T], BF16, tag=f"KT{p}", name=f"KT{p}") for p in range(4)]
            Y2 = [sb.tile([P, T], BF16, tag=f"Y{p}", name=f"Y{p}") for p in range(4)]
            V_sb = sb.tile([P, 16, HPC, 65], BF16, tag="Vsb")
            tTq = sb.tile([P, T], BF16, tag="tTq")
            tTk = sb.tile([P, T], BF16, tag="tTk")
            tTv = sb.tile([P, T], BF16, tag="tTv")
            rT_sb = sb.tile([P, T], BF16, tag="rT")
            onesP = sb.tile([65, 64], BF16, tag="onesP")
            den_t = [sb.tile([65, 4096], BF16, tag=f"den{p}", name=f"den{p}")
                     for p in range(4)]

            nc.gpsimd.memset(V_sb[:, :, :, 64:65], 1.0)
            nc.gpsimd.memset(onesP, 1.0)
            # spin the PE during the input-DMA window so HAM un-throttles the
            # clock (1.2 -> 2.4 GHz) before stage 1 begins
            with tc.tile_pool(name="psW", bufs=1, space="PSUM") as psW:
                pw = psW.tile([64, 512], F32, tag="pw")
                for _ in range(12):
                    nc.tensor.matmul(
                        pw, warm_sb[:, 0:64], warm_sb, start=True, stop=True)

            # ---- phase A: t^T = A @ x^T for q,k (interleaved), then v ----
            with tc.tile_pool(name="psA", bufs=2, space="PSUM") as psA:
                ptq = psA.tile([P, T], F32, tag="psA", name="ptq")
                ptk = psA.tile([P, T], F32, tag="psA", name="ptk")
                for cc in range(8):
                    for AT_sb, pt in ((qAT_sb, ptq), (kAT_sb, ptk)):
                        for t0, tw in _chunks(T, 512):
                            nc.tensor.matmul(
                                pt[:, t0:t0 + tw],
                                AT_sb[:, cc, :],
                                xT_sb[:, cc, t0:t0 + tw],
                                start=(cc == 0), stop=(cc == 7),
                            )
                nc.vector.tensor_copy(out=tTq, in_=ptq)
                nc.vector.tensor_copy(out=tTk, in_=ptk)
                ptv = psA.tile([P, T], F32, tag="psA", name="ptv")
                for cc in range(8):
                    for t0, tw in _chunks(1024, 512):
                        nc.tensor.matmul(
                            ptv[:, t0:t0 + tw],
                            vAT_sb[:, cc, :],
                            xT_sb[:, cc, t0:t0 + tw],
                            start=(cc == 0), stop=(cc == 7),
                        )
                nc.vector.tensor_copy(out=tTv[:, 0:1024], in_=ptv[:, 0:1024])

            # ---- phase B: paired Q^T/K^T (M=128) and keys-major V ----
            with (
                tc.tile_pool(name="psB", bufs=2, space="PSUM") as psB,
                tc.tile_pool(name="psV", bufs=2, space="PSUM") as psV,
            ):
                def stage2_pair(hp):
                    for BT_sb, dest, tT in ((qBT_sb, QT2, tTq), (kBT_sb, KT2, tTk)):
                        for t0, tw in _chunks(T, 512):
                            p2 = psB.tile([P, 512], F32, tag="psB")
                            nc.tensor.matmul(
                                p2[:, :tw],
                                BT_sb[:, hp * P:(hp + 1) * P],
                                tT[:, t0:t0 + tw],
                                start=True, stop=True,
                            )
                            nc.vector.tensor_copy(
                                out=dest[hp][:, t0:t0 + tw], in_=p2[:, :tw])

                stage2_pair(0)
                for ti in range(8):
                    pv = psV.tile([P, G], F32, tag="psV")
                    nc.tensor.matmul(
                        pv, tTv[:, ti * 128:(ti + 1) * 128], vBT_sb,
                        start=True, stop=True,
                    )
                    nc.vector.tensor_copy(
                        out=V_sb[:, ti, :, 0:64],
                        in_=pv.rearrange("p (h d) -> p h d", d=64),
                    )

            # ---- attention + deferred normalize ----
            with (
                tc.tile_pool(name="psS", bufs=2, space="PSUM") as psS,
                tc.tile_pool(name="psPV", bufs=3, space="PSUM") as psPV,
                tc.tile_pool(name="ptp", bufs=4) as ptp,
            ):
                strips = [(hp, j, kb)
                          for hp in range(4)
                          for j in range(4)
                          for kb in range(4 * j + 4)]
                pvt = {}
                sps_l = {}

                def stage2_pair_att(hp2):
                    for BT_sb, dest, tT in ((qBT_sb, QT2, tTq), (kBT_sb, KT2, tTk)):
                        for t0, tw in _chunks(T, 512):
                            p2 = psS.tile([P, 512], F32, tag="s2", bufs=1)
                            nc.tensor.matmul(
                                p2[:, :tw],
                                BT_sb[:, hp2 * P:(hp2 + 1) * P],
                                tT[:, t0:t0 + tw],
                                start=True, stop=True,
                            )
                            nc.vector.tensor_copy(
                                out=dest[hp2][:, t0:t0 + tw], in_=p2[:, :tw])

                def v_chunk_att(c):
                    pvc = psS.tile([P, 512], F32, tag="s2", bufs=1, name=f"pvc{c}")
                    for cc in range(8):
                        nc.tensor.matmul(
                            pvc, vAT_sb[:, cc, :],
                            xT_sb[:, cc, 512 * c:512 * (c + 1)],
                            start=(cc == 0), stop=(cc == 7),
                        )
                    nc.vector.tensor_copy(
                        out=tTv[:, 512 * c:512 * (c + 1)], in_=pvc)

                def v_mm_att(ti):
                    pv = psS.tile([P, G], F32, tag="s2", bufs=1, name=f"pvv{ti}")
                    nc.tensor.matmul(
                        pv, tTv[:, ti * 128:(ti + 1) * 128], vBT_sb,
                        start=True, stop=True,
                    )
                    nc.vector.tensor_copy(
                        out=V_sb[:, ti, :, 0:64],
                        in_=pv.rearrange("p (h d) -> p h d", d=64),
                    )

                att_work = {
                    1: lambda: v_chunk_att(2),
                    3: lambda: (v_mm_att(8), v_mm_att(9)),
                    4: lambda: (v_mm_att(10), v_mm_att(11)),
                    6: lambda: v_chunk_att(3),
                    8: lambda: (v_mm_att(12), v_mm_att(13)),
                    9: lambda: (v_mm_att(14), v_mm_att(15)),
                    14: lambda: stage2_pair_att(1),
                    30: lambda: stage2_pair_att(2),
                    46: lambda: stage2_pair_att(3),
                }
                for idx in range(len(strips) + 1):
                    if idx in att_work:
                        att_work[idx]()
                    if idx < len(strips):
                        # emit paired QK for strip idx ahead (row-tiled: both
                        # heads of the pair run concurrently in the PE array)
                        hp, j, kb = strips[idx]
                        c0 = 128 * (kb - 4 * j) if kb >= 4 * j else 0
                        w = 512 - c0
                        qlo = 512 * j + c0
                        sps = psS.tile([P, 2, 512], F32, tag="s")
                        for par in range(2):
                            rws = slice(64 * par, 64 * par + 64)
                            nc.tensor.matmul(
                                sps[:, par, c0:c0 + w],
                                KT2[hp][rws, kb * 128:(kb + 1) * 128],
                                QT2[hp][rws, qlo:qlo + w],
                                start=True, stop=True,
                            )
                        sps_l[idx] = sps
                    if idx == 0:
                        continue
                    hp, j, kb = strips[idx - 1]
                    c0 = 128 * (kb - 4 * j) if kb >= 4 * j else 0
                    w = 512 - c0
                    sps = sps_l.pop(idx - 1)
                    if kb == 0:
                        for par in range(2):
                            pvt[(hp, par, j)] = psPV.tile(
                                [65, 512], F32, tag="pv", name=f"pv{hp}_{par}_{j}")
                    ptile = ptp.tile([P, 2, 512], BF16, tag="pt")
                    nc.scalar.activation(
                        ptile[:, :, c0:], sps[:, :, c0:], Exp, scale=0.125)
                    for par in range(2):
                        h = 2 * hp + par
                        rows = slice(64 * par, 64 * par + 64)
                        if kb >= 4 * j:  # zero masked diag region post-exp
                            nc.vector.tensor_tensor(
                                out=ptile[:, par, c0:c0 + P],
                                in0=ptile[:, par, c0:c0 + P],
                                in1=mask_sb, op=mybir.AluOpType.mult,
                            )
                        nc.tensor.matmul(
                            pvt[(hp, par, j)][:, c0:],
                            V_sb[:, kb, h, :],
                            ptile[:, par, c0:],
                            start=(kb == 0), stop=(kb == 4 * j + 3),
                        )
                    for par in range(2):
                        if kb == 4 * j + 3:
                            rows = slice(64 * par, 64 * par + 64)
                            r0 = 512 * j
                            pv_t = pvt.pop((hp, par, j))
                            doff = 2048 * par + 512 * j
                            # denominator -> SBUF bf16 (recip deferred to tail)
                            nc.vector.tensor_copy(
                                out=den_t[hp][64:65, doff:doff + 512],
                                in_=pv_t[64:65, :],
                            )
                            # unnormalized Y' -> bf16 SBUF (frees psum)
                            nc.vector.tensor_copy(
                                out=Y2[hp][rows, r0:r0 + 512],
                                in_=pv_t[0:64, :],
                            )

            # ---- normalize tail: ScalarE reciprocal (one table switch),
            # ones-matmul broadcast, DVE multiply ----
            with (
                tc.tile_pool(name="psN", bufs=2, space="PSUM") as psN,
                tc.tile_pool(name="nr2", bufs=3) as nr2,
                tc.tile_pool(name="psD", bufs=1, space="PSUM") as psD,
            ):
                # all 8 reciprocals back-to-back on ScalarE (one table switch)
                rec_l = {}
                for hp in range(4):
                    for par in range(2):
                        doff = 2048 * par
                        rec_t = nr2.tile([65, T], BF16, tag="rect")
                        _scalar_recip(
                            nc, rec_t[64:65, :],
                            den_t[hp][64:65, doff:doff + T],
                        )
                        rec_l[(hp, par)] = rec_t
                pr = psD.tile([P, T], F32, tag="r")
                for hp in range(4):
                    for par in range(2):
                        rows = slice(64 * par, 64 * par + 64)
                        rec_t = rec_l.pop((hp, par))
                        for half in range(2):
                            h0 = 1024 * half
                            bcT = psN.tile([64, 1024], F32, tag="bc", bufs=2)
                            for r0, rw in _chunks(1024, 512):
                                nc.tensor.matmul(
                                    bcT[:, r0:r0 + rw], onesP[64:65, :],
                                    rec_t[64:65, h0 + r0:h0 + r0 + rw],
                                    start=True, stop=True,
                                )
                            nc.vector.tensor_tensor(
                                out=Y2[hp][rows, h0:h0 + 1024],
                                in0=Y2[hp][rows, h0:h0 + 1024],
                                in1=bcT, op=mybir.AluOpType.mult,
                            )
                    # c_proj stage 1 for this pair rides right behind its
                    # normalize (accumulates across pairs in PSUM)
                    for t0, tw in _chunks(T, 512):
                        nc.tensor.matmul(
                            pr[:, t0:t0 + tw], cAT2_sb[:, hp, :], Y2[hp][:, t0:t0 + tw],
                            start=(hp == 0), stop=(hp == 3),
                        )

                nc.vector.tensor_copy(out=rT_sb, in_=pr)

            # ---- phase D2: c_proj output, ti-pair pipelined ----
            out_r = out.rearrange("(tt pp) c -> pp tt c", pp=P)
            with (
                tc.tile_pool(name="psO", bufs=2, space="PSUM") as psO,
                tc.tile_pool(name="ost", bufs=3) as ost,
            ):
                for tp in range(8):
                    po = psO.tile([P, 2, 2, 512], F32, tag="o")
                    for tt in range(2):
                        ti = 2 * tp + tt
                        for nn in range(2):
                            nc.tensor.matmul(
                                po[:, tt, nn, :], rT_sb[:, ti * 128:(ti + 1) * 128],
                                cBT_sb[:, nn * 512:(nn + 1) * 512],
                                start=True, stop=True,
                            )
                    ob = ost.tile([P, 2, C], BF16, tag="ob")
                    if tp % 2 == 0:
                        nc.vector.tensor_copy(out=ob, in_=po)
                    else:
                        nc.scalar.copy(ob, po)
                    q = nc.sync if tp % 2 == 0 else nc.gpsimd
                    q.dma_start(
                        out=out_r[:, 2 * tp:2 * tp + 2, :],
                        in_=ob,
                    )
    nc.finalize()
    return nc


def make_in_maps(x, qA, qB, kA, kB, vA, vB, cA, cB):
    x, qA, qB, kA, kB, vA, vB, cA, cB = [
        np.asarray(a, dtype=np.float32) for a in (x, qA, qB, kA, kB, vA, vB, cA, cB)
    ]
    mask = np.where(
        np.arange(P)[:, None] <= np.arange(P)[None, :], 1.0, 0.0
    ).astype(BF)
    qATn = np.ascontiguousarray(qA.T).astype(BF)
    kATn = np.ascontiguousarray(kA.T).astype(BF)
    vATn = np.ascontiguousarray(vA.T).astype(BF)
    cBTn = np.ascontiguousarray(cB.T).astype(BF)
    in_maps = []
    for c in range(NCORES):
        b, g = divmod(c, 2)
        sl = slice(g * G, (g + 1) * G)
        in_maps.append({
            "xT": np.ascontiguousarray(x[b].T).astype(BF),
            "qAT": qATn, "kAT": kATn, "vAT": vATn,
            "qBT": np.ascontiguousarray(qB[sl, :].T).astype(BF),
            "kBT": np.ascontiguousarray(kB[sl, :].T).astype(BF),
            "vBT": np.ascontiguousarray(vB[sl, :].T).astype(BF),
            "cAT": np.ascontiguousarray(cA[:, sl].T).astype(BF),
            "cBT": cBTn,
            "mask": mask,
        })
    return in_maps


def combine(parts):
    return np.stack(
        [parts[2 * b].astype(np.float32) + parts[2 * b + 1].astype(np.float32)
         for b in range(B)], axis=0)


def kernel(x, qA, qB, kA, kB, vA, vB, cA, cB):
    global _NC_CACHE
    if _NC_CACHE is None:
        _NC_CACHE = build()
    in_maps = make_in_maps(x, qA, qB, kA, kB, vA, vB, cA, cB)
    res = run_bass_kernel_spmd(_NC_CACHE, in_maps, list(range(NCORES))).results
    return combine([res[c]["out"] for c in range(NCORES)])
